# revision 5
# baseline (speedup 1.0000x reference)
import sys, time
if "/opt/trn_rl_repo" not in sys.path:
    sys.path.insert(0, "/opt/trn_rl_repo")
import numpy as np
import concourse.bass as bass
import concourse.bacc as bacc
from concourse import tile
from concourse.bass_utils import run_bass_kernel_spmd

mybir = bass.mybir

NC = 8
U, I, B = 100000, 50000, 20000
D = 64
UPC, IPC, BPC = U // NC, I // NC, B // NC          # 12500, 6250, 2500
UPAD, IPAD, BPAD = 12544, 6272, 2560               # multiples of 128
NLOC = UPAD + IPAD                                 # 18816 local rows per core
NWIN = NLOC // 128                                 # 147
UWIN = UPAD // 128                                 # 98
XROWS = NC * NLOC                                  # 150528 perm-table rows
BROWS = NC * BPAD                                  # 20480
ITEM_AUG_ROWS = 50176
BUCKET = 32768
CHUNK = 8192

last_exec_wall = None


def _perm_node(g):
    g = np.asarray(g, np.int64)
    is_item = g >= U
    u = np.minimum(g, U - 1)
    ku = u // UPC
    pu = ku * NLOC + (u - ku * UPC)
    it = np.maximum(g - U, 0)
    ki = it // IPC
    pi = ki * NLOC + UPAD + (it - ki * IPC)
    return np.where(is_item, pi, pu)


def _wrap_idx(a):
    return np.tile(a.reshape(-1, 16).T, (8, 1)).astype(np.int16)


def _calls(slots):
    out, off = [], 0
    while off < slots:
        c = min(CHUNK, slots - off)
        out.append((off, c))
        off += c
    return out


class _Plan:
    """Static spmm schedule, identical across cores.
    T[b][w] = columns for (bucket, window)."""
    def __init__(self, nbuck, nwin, dout, T):
        self.nbuck, self.nwin, self.dout, self.T = nbuck, nwin, dout, T
        self.bslots = [int(T[b].sum()) * 128 for b in range(nbuck)]
        self.calls = [_calls(s) for s in self.bslots]
        self.colwin = [np.repeat(np.arange(nwin), T[b]) for b in range(nbuck)]


def _schedule(core, dl, cp, vv, nbuck, nwin, dout):
    """Build shared schedule + per-core slot streams."""
    key = (cp // BUCKET) * nwin + (dl // 128)
    counts = np.zeros((NC, nbuck * nwin), np.int64)
    for k in range(NC):
        counts[k] = np.bincount(key[core == k], minlength=nbuck * nwin)
    T = -(-counts.max(0) // 128)          # ceil(max/128), may be 0
    T = T.reshape(nbuck, nwin)
    plan = _Plan(nbuck, nwin, dout, T)
    gstart = np.concatenate([[0], np.cumsum(T.reshape(-1) * 128)])[:-1]

    streams = []
    for k in range(NC):
        m = core == k
        dlk, cpk, vvk = dl[m], cp[m], vv[m]
        keyk = (cpk // BUCKET) * nwin + (dlk // 128)
        o = np.argsort(keyk, kind="stable")
        dlk, cpk, vvk, keyk = dlk[o], cpk[o], vvk[o], keyk[o]
        ck = counts[k]
        ranks = np.arange(len(keyk)) - np.repeat(np.concatenate([[0], np.cumsum(ck)])[:-1], ck)
        slots = gstart[keyk] + ranks
        tot = int(T.sum()) * 128
        idx = np.zeros(tot, np.int16)
        dstr = np.zeros(tot, np.float32)
        val = np.zeros(tot, np.float32)
        idx[slots] = (cpk % BUCKET).astype(np.int16)
        dstr[slots] = (dlk % 128).astype(np.float32)
        val[slots] = vvk
        streams.append((idx, dstr, val))
    return plan, streams


def _emit_spmm(nc, sb, ps, plan, table_ap, table_rows, ix_dram, dv_dram, vl_dram,
               out_sb, iota0, tagp):
    for b in range(plan.nbuck):
        if plan.bslots[b] == 0:
            continue
        base_rows = b * BUCKET
        nrows = min(BUCKET, table_rows - base_rows)
        colwin = plan.colwin[b]
        ncol_total = plan.bslots[b] // 128
        first = {}
        last = {}
        for j in range(ncol_total):
            w = int(colwin[j])
            first.setdefault(w, j)
            last[w] = j
        ptile = {}
        for ci, (off, csz) in enumerate(plan.calls[b]):
            ccols = csz // 128
            ix = sb.tile([128, CHUNK // 16], mybir.dt.int16, tag="ix")
            nc.sync.dma_start(ix[:, :csz // 16], ix_dram[b, ci, :, :csz // 16])
            gat = sb.tile([128, CHUNK // 128, plan.dout], mybir.dt.float32, tag="g")
            nc.gpsimd.dma_gather(gat[:, :ccols, :],
                                 table_ap[base_rows:base_rows + nrows, :],
                                 ix[:, :csz // 16], csz, csz, plan.dout,
                                 single_packet=False)
            dv = sb.tile([128, CHUNK // 128], mybir.dt.float32, tag="d")
            vl = sb.tile([128, CHUNK // 128], mybir.dt.float32, tag="v")
            nc.sync.dma_start(dv[:, :ccols], dv_dram[b, ci, :, :ccols])
            nc.sync.dma_start(vl[:, :ccols], vl_dram[b, ci, :, :ccols])
            for lc in range(ccols):
                j = off // 128 + lc
                w = int(colwin[j])
                S = sb.tile([128, 128], mybir.dt.float32, tag="S")
                nc.vector.tensor_scalar(S[:], iota0[:], dv[:, lc:lc + 1],
                                        vl[:, lc:lc + 1],
                                        mybir.AluOpType.is_equal,
                                        mybir.AluOpType.mult)
                if w not in ptile:
                    ptile[w] = ps.tile([128, plan.dout], mybir.dt.float32,
                                       tag="p", name=f"pw{tagp}_{w}")
                nc.tensor.matmul(ptile[w][:], lhsT=S[:], rhs=gat[:, lc, :],
                                 start=(first[w] == j), stop=(last[w] == j))
                if last[w] == j:
                    nc.vector.tensor_add(out_sb[:, w, :], out_sb[:, w, :],
                                         ptile[w][:])
                    del ptile[w]
        assert not ptile


def _build_and_run(in_maps, adj_plan, hv_plan, hu_plan):
    global last_exec_wall
    nc = bacc.Bacc("TRN2", target_bir_lowering=False, debug=False, num_devices=NC)
    f32, i16 = mybir.dt.float32, mybir.dt.int16

    x0 = nc.dram_tensor("x0", [XROWS, D], f32, kind="ExternalInput")
    x0sh = nc.dram_tensor("x0sh", [128, NWIN, D], f32, kind="ExternalInput")
    item_aug = nc.dram_tensor("item_aug", [ITEM_AUG_ROWS, 128], f32, kind="ExternalInput")

    def edge_inputs(pfx, plan):
        ncall = max(len(c) for c in plan.calls)
        ixs = nc.dram_tensor(f"{pfx}_ix", [plan.nbuck, ncall, 128, CHUNK // 16], i16,
                             kind="ExternalInput")
        dvs = nc.dram_tensor(f"{pfx}_dv", [plan.nbuck, ncall, 128, CHUNK // 128], f32,
                             kind="ExternalInput")
        vls = nc.dram_tensor(f"{pfx}_vl", [plan.nbuck, ncall, 128, CHUNK // 128], f32,
                             kind="ExternalInput")
        return ixs, dvs, vls

    adj_io = edge_inputs("adj", adj_plan)
    hv_io = edge_inputs("hv", hv_plan)
    hu_io = edge_inputs("hu", hu_plan)

    out_u = nc.dram_tensor("out_u", [128, UWIN, D], f32, kind="ExternalOutput")
    out_i = nc.dram_tensor("out_i", [128, IPAD // 128, D], f32, kind="ExternalOutput")

    xf1 = nc.dram_tensor("xf1", [XROWS, D], f32, addr_space="Shared")
    xf2 = nc.dram_tensor("xf2", [XROWS, D], f32, addr_space="Shared")
    ebounce = nc.dram_tensor("ebounce", [NLOC, D], f32)
    bbounce = nc.dram_tensor("bbounce", [BPAD, 128], f32)
    bfull = nc.dram_tensor("bfull", [BROWS, 128], f32, addr_space="Shared")

    with tile.TileContext(nc) as tc:
        with tc.tile_pool(name="persist", bufs=1) as persist, \
             tc.tile_pool(name="ps", bufs=8, space="PSUM") as ps:
            iota0 = persist.tile([128, 128], f32)
            nc.gpsimd.iota(iota0[:], pattern=[[1, 128]], base=0, channel_multiplier=0,
                           allow_small_or_imprecise_dtypes=True)
            acc = persist.tile([128, NWIN, D], f32)
            nc.sync.dma_start(acc[:], x0sh[:])

            with tc.tile_pool(name="adjbig", bufs=1) as adjbig, \
                 tc.tile_pool(name="sb1", bufs=3) as sb:
                out_sb = adjbig.tile([128, NWIN, D], f32)
                for li, srct in enumerate([x0, xf1, xf2]):
                    nc.vector.memset(out_sb[:], 0.0)
                    _emit_spmm(nc, sb, ps, adj_plan, srct, XROWS, *adj_io,
                               out_sb, iota0, f"a{li}")
                    nc.vector.tensor_add(acc[:], acc[:], out_sb[:])
                    if li < 2:
                        nc.sync.dma_start(ebounce.rearrange("(n p) d -> p n d", p=128),
                                          out_sb[:])
                        nc.gpsimd.collective_compute(
                            "AllGather", mybir.AluOpType.bypass,
                            replica_groups=[list(range(NC))],
                            ins=[ebounce[:].opt()], outs=[[xf1, xf2][li][:].opt()])

            with tc.tile_pool(name="hubig", bufs=1) as hubig, \
                 tc.tile_pool(name="sb2", bufs=2) as sb:
                # ---- hv: items -> bicliques (aug col 64 = degree)
                bic = hubig.tile([128, BPAD // 128, 128], f32)
                nc.vector.memset(bic[:], 0.0)
                _emit_spmm(nc, sb, ps, hv_plan, item_aug, ITEM_AUG_ROWS, *hv_io,
                           bic, iota0, "hv")
                nbr = BPAD // 128
                deg = sb.tile([128, nbr], f32, tag="bdeg")
                rec = sb.tile([128, nbr], f32, tag="brec")
                nc.vector.scalar_tensor_tensor(deg[:], bic[:, :, 64], 0.0, bic[:, :, 64],
                                               mybir.AluOpType.is_equal,
                                               mybir.AluOpType.add)
                nc.vector.reciprocal(rec[:], deg[:])
                for r in range(nbr):
                    nc.vector.tensor_scalar_mul(bic[:, r, 0:64], bic[:, r, 0:64],
                                                rec[:, r:r + 1])
                nc.vector.memset(bic[:, :, 64:65], 1.0)
                nc.vector.memset(bic[:, :, 65:128], 0.0)
                nc.sync.dma_start(bbounce.rearrange("(n p) d -> p n d", p=128), bic[:])
                nc.gpsimd.collective_compute(
                    "AllGather", mybir.AluOpType.bypass,
                    replica_groups=[list(range(NC))],
                    ins=[bbounce[:].opt()], outs=[bfull[:].opt()])

                # ---- hu: bicliques -> users
                ul = hubig.tile([128, UWIN, 128], f32)
                nc.vector.memset(ul[:], 0.0)
                _emit_spmm(nc, sb, ps, hu_plan, bfull, BROWS, *hu_io,
                           ul, iota0, "hu")
                udeg = sb.tile([128, UWIN], f32, tag="udeg")
                urec = sb.tile([128, UWIN], f32, tag="urec")
                nc.vector.scalar_tensor_tensor(udeg[:], ul[:, :, 64], 0.0, ul[:, :, 64],
                                               mybir.AluOpType.is_equal,
                                               mybir.AluOpType.add)
                nc.vector.reciprocal(urec[:], udeg[:])
                nc.vector.tensor_scalar(acc[:], acc[:], 0.25, None,
                                        mybir.AluOpType.mult)
                for r in range(UWIN):
                    nc.vector.tensor_scalar(ul[:, r, 0:64], ul[:, r, 0:64],
                                            urec[:, r:r + 1], None,
                                            mybir.AluOpType.mult)
                    nc.vector.tensor_add(ul[:, r, 0:64], ul[:, r, 0:64],
                                         acc[:, r, :])
                nc.sync.dma_start(out_u[:], ul[:, :, 0:64])
                nc.sync.dma_start(out_i[:], acc[:, UWIN:NWIN, :])
    nc.compile()
    t0 = time.time()
    res = run_bass_kernel_spmd(nc, in_maps, list(range(NC)))
    last_exec_wall = time.time() - t0
    return res


def _pack_inputs(stream, plan):
    ncall = max(len(c) for c in plan.calls)
    ix = np.zeros((plan.nbuck, ncall, 128, CHUNK // 16), np.int16)
    dv = np.zeros((plan.nbuck, ncall, 128, CHUNK // 128), np.float32)
    vl = np.zeros((plan.nbuck, ncall, 128, CHUNK // 128), np.float32)
    idx, dstr, val = stream
    boff = 0
    for b in range(plan.nbuck):
        for ci, (off, csz) in enumerate(plan.calls[b]):
            s = boff + off
            ix[b, ci, :, :csz // 16] = _wrap_idx(idx[s:s + csz])
            dv[b, ci, :, :csz // 128] = dstr[s:s + csz].reshape(-1, 128).T
            vl[b, ci, :, :csz // 128] = val[s:s + csz].reshape(-1, 128).T
        boff += plan.bslots[b]
    return ix, dv, vl


def kernel(user_emb, item_emb, adj_val, hv_val, hu_val,
           adj_row, adj_col, hv_row, hv_col, hu_row, hu_col):
    user_emb = np.asarray(user_emb, np.float32)
    item_emb = np.asarray(item_emb, np.float32)
    adj_val = np.asarray(adj_val, np.float32)
    hv_val = np.asarray(hv_val, np.float32)
    hu_val = np.asarray(hu_val, np.float32)
    adj_row = np.asarray(adj_row, np.int64)
    adj_col = np.asarray(adj_col, np.int64)
    hv_row = np.asarray(hv_row, np.int64)
    hv_col = np.asarray(hv_col, np.int64)
    hu_row = np.asarray(hu_row, np.int64)
    hu_col = np.asarray(hu_col, np.int64)

    x0 = np.zeros((XROWS, D), np.float32)
    allp = _perm_node(np.arange(U + I))
    x0[allp[:U]] = user_emb
    x0[allp[U:]] = item_emb

    item_aug = np.zeros((ITEM_AUG_ROWS, 128), np.float32)
    item_aug[:I, :64] = item_emb
    item_aug[:I, 64] = 1.0

    def core_of_node(g):
        g = np.asarray(g, np.int64)
        return np.where(g < U, np.minimum(g // UPC, NC - 1),
                        np.minimum((g - U) // IPC, NC - 1))

    def local_of_node(g):
        g = np.asarray(g, np.int64)
        k = core_of_node(g)
        return np.where(g < U, g - k * UPC, UPAD + (g - U) - k * IPC)

    adj_core = core_of_node(adj_row)
    adj_dl = local_of_node(adj_row)
    adj_cp = _perm_node(adj_col)
    hv_core = hv_row // BPC
    hv_dl = hv_row - hv_core * BPC
    hu_core = hu_row // UPC
    hu_dl = hu_row - hu_core * UPC
    hu_cp = hu_col // BPC * BPAD + hu_col % BPC

    adj_plan, adj_streams = _schedule(adj_core, adj_dl, adj_cp, adj_val, 5, NWIN, D)
    hv_plan, hv_streams = _schedule(hv_core, hv_dl, hv_col, hv_val, 2, BPAD // 128, 128)
    hu_plan, hu_streams = _schedule(hu_core, hu_dl, hu_cp, hu_val, 1, UWIN, 128)

    in_maps = []
    for k in range(NC):
        a = _pack_inputs(adj_streams[k], adj_plan)
        v = _pack_inputs(hv_streams[k], hv_plan)
        u = _pack_inputs(hu_streams[k], hu_plan)
        x0sh = x0[k * NLOC:(k + 1) * NLOC].reshape(NWIN, 128, D).transpose(1, 0, 2).copy()
        in_maps.append({
            "x0": x0, "x0sh": x0sh, "item_aug": item_aug,
            "adj_ix": a[0], "adj_dv": a[1], "adj_vl": a[2],
            "hv_ix": v[0], "hv_dv": v[1], "hv_vl": v[2],
            "hu_ix": u[0], "hu_dv": u[1], "hu_vl": u[2],
        })

    res = _build_and_run(in_maps, adj_plan, hv_plan, hu_plan)

    u_out = np.zeros((U, D), np.float32)
    i_out = np.zeros((I, D), np.float32)
    for k in range(NC):
        ou = res.results[k]["out_u"]
        oi = res.results[k]["out_i"]
        u_out[k * UPC:(k + 1) * UPC] = ou.transpose(1, 0, 2).reshape(UPAD, D)[:UPC]
        i_out[k * IPC:(k + 1) * IPC] = oi.transpose(1, 0, 2).reshape(IPAD, D)[:IPC]
    return u_out, i_out


# revision 6
# speedup vs baseline: 5713.5372x; 5713.5372x over previous
import sys, time
if "/opt/trn_rl_repo" not in sys.path:
    sys.path.insert(0, "/opt/trn_rl_repo")
import numpy as np
import concourse.bass as bass
import concourse.bacc as bacc
from concourse import tile
from concourse.bass_utils import run_bass_kernel_spmd

mybir = bass.mybir

NC = 8
U, I, B = 100000, 50000, 20000
D = 64
UPC, IPC, BPC = U // NC, I // NC, B // NC          # 12500, 6250, 2500
UPAD, IPAD, BPAD = 12544, 6272, 2560               # multiples of 128
NLOC = UPAD + IPAD                                 # 18816 local rows per core
NWIN = NLOC // 128                                 # 147
UWIN = UPAD // 128                                 # 98
XROWS = NC * NLOC                                  # 150528 perm-table rows
BROWS = NC * BPAD                                  # 20480
ITEM_AUG_ROWS = 50176
BUCKET = 32768
CHUNK = 8192

last_exec_wall = None
last_exec_ns = None


def _install_ntff_hook():
    """Provide antenv.axon_hooks (absent in this image) so
    run_bass_kernel_spmd(trace=True) can NTFF-profile via libaxon_pjrt."""
    import types, ctypes, contextlib, importlib
    try:
        import antenv.axon_hooks  # noqa
        return True
    except ImportError:
        pass
    try:
        lib = ctypes.CDLL("/opt/axon/libaxon_pjrt.so")
        if not hasattr(lib, "axon_start_nrt_profile"):
            return False
        lib.axon_start_nrt_profile.argtypes = [ctypes.POINTER(ctypes.c_int64), ctypes.c_size_t]
        lib.axon_start_nrt_profile.restype = ctypes.c_int64
        lib.axon_stop_nrt_profile.argtypes = [ctypes.c_char_p]
        lib.axon_stop_nrt_profile.restype = ctypes.c_int64

        @contextlib.contextmanager
        def _hook(output_dir, device_ids):
            import jax
            jax.devices()
            if device_ids:
                ids = (ctypes.c_int64 * len(device_ids))(*device_ids)
                rc = lib.axon_start_nrt_profile(ids, len(device_ids))
            else:
                rc = lib.axon_start_nrt_profile(None, 0)
            try:
                yield
            finally:
                if rc == 0:
                    lib.axon_stop_nrt_profile(output_dir.encode())

        mod = types.ModuleType("antenv.axon_hooks")
        _state = {"hook": _hook}
        mod.get_axon_ntff_profile_hook = lambda: _state["hook"]
        mod.set_axon_ntff_profile_hook = lambda h: _state.update(hook=h)
        import antenv
        sys.modules["antenv.axon_hooks"] = mod
        antenv.axon_hooks = mod
        return True
    except Exception:
        return False


def _perm_node(g):
    g = np.asarray(g, np.int64)
    is_item = g >= U
    u = np.minimum(g, U - 1)
    ku = u // UPC
    pu = ku * NLOC + (u - ku * UPC)
    it = np.maximum(g - U, 0)
    ki = it // IPC
    pi = ki * NLOC + UPAD + (it - ki * IPC)
    return np.where(is_item, pi, pu)


def _wrap_idx(a):
    return np.tile(a.reshape(-1, 16).T, (8, 1)).astype(np.int16)


def _calls(slots):
    out, off = [], 0
    while off < slots:
        c = min(CHUNK, slots - off)
        out.append((off, c))
        off += c
    return out


class _Plan:
    """Static spmm schedule, identical across cores.
    T[b][w] = columns for (bucket, window)."""
    def __init__(self, nbuck, nwin, dout, T):
        self.nbuck, self.nwin, self.dout, self.T = nbuck, nwin, dout, T
        self.bslots = [int(T[b].sum()) * 128 for b in range(nbuck)]
        self.calls = [_calls(s) for s in self.bslots]
        self.colwin = [np.repeat(np.arange(nwin), T[b]) for b in range(nbuck)]


def _schedule(core, dl, cp, vv, nbuck, nwin, dout):
    """Build shared schedule + per-core slot streams."""
    key = (cp // BUCKET) * nwin + (dl // 128)
    counts = np.zeros((NC, nbuck * nwin), np.int64)
    for k in range(NC):
        counts[k] = np.bincount(key[core == k], minlength=nbuck * nwin)
    T = -(-counts.max(0) // 128)          # ceil(max/128), may be 0
    T = T.reshape(nbuck, nwin)
    plan = _Plan(nbuck, nwin, dout, T)
    gstart = np.concatenate([[0], np.cumsum(T.reshape(-1) * 128)])[:-1]

    streams = []
    for k in range(NC):
        m = core == k
        dlk, cpk, vvk = dl[m], cp[m], vv[m]
        keyk = (cpk // BUCKET) * nwin + (dlk // 128)
        o = np.argsort(keyk, kind="stable")
        dlk, cpk, vvk, keyk = dlk[o], cpk[o], vvk[o], keyk[o]
        ck = counts[k]
        ranks = np.arange(len(keyk)) - np.repeat(np.concatenate([[0], np.cumsum(ck)])[:-1], ck)
        slots = gstart[keyk] + ranks
        tot = int(T.sum()) * 128
        idx = np.zeros(tot, np.int16)
        dstr = np.zeros(tot, np.float32)
        val = np.zeros(tot, np.float32)
        idx[slots] = (cpk % BUCKET).astype(np.int16)
        dstr[slots] = (dlk % 128).astype(np.float32)
        val[slots] = vvk
        streams.append((idx, dstr, val))
    return plan, streams


def _emit_spmm(nc, sb, ps, plan, table_ap, table_rows, ix_dram, dv_dram, vl_dram,
               out_sb, iota0, tagp):
    for b in range(plan.nbuck):
        if plan.bslots[b] == 0:
            continue
        base_rows = b * BUCKET
        nrows = min(BUCKET, table_rows - base_rows)
        colwin = plan.colwin[b]
        ncol_total = plan.bslots[b] // 128
        first = {}
        last = {}
        for j in range(ncol_total):
            w = int(colwin[j])
            first.setdefault(w, j)
            last[w] = j
        ptile = {}
        for ci, (off, csz) in enumerate(plan.calls[b]):
            ccols = csz // 128
            ix = sb.tile([128, CHUNK // 16], mybir.dt.int16, tag="ix")
            nc.sync.dma_start(ix[:, :csz // 16], ix_dram[b, ci, :, :csz // 16])
            gat = sb.tile([128, CHUNK // 128, plan.dout], mybir.dt.float32, tag="g")
            nc.gpsimd.dma_gather(gat[:, :ccols, :],
                                 table_ap[base_rows:base_rows + nrows, :],
                                 ix[:, :csz // 16], csz, csz, plan.dout,
                                 single_packet=False)
            dv = sb.tile([128, CHUNK // 128], mybir.dt.float32, tag="d")
            vl = sb.tile([128, CHUNK // 128], mybir.dt.float32, tag="v")
            nc.sync.dma_start(dv[:, :ccols], dv_dram[b, ci, :, :ccols])
            nc.sync.dma_start(vl[:, :ccols], vl_dram[b, ci, :, :ccols])
            for lc in range(ccols):
                j = off // 128 + lc
                w = int(colwin[j])
                S = sb.tile([128, 128], mybir.dt.float32, tag="S")
                nc.vector.tensor_scalar(S[:], iota0[:], dv[:, lc:lc + 1],
                                        vl[:, lc:lc + 1],
                                        mybir.AluOpType.is_equal,
                                        mybir.AluOpType.mult)
                if w not in ptile:
                    ptile[w] = ps.tile([128, plan.dout], mybir.dt.float32,
                                       tag="p", name=f"pw{tagp}_{w}")
                nc.tensor.matmul(ptile[w][:], lhsT=S[:], rhs=gat[:, lc, :],
                                 start=(first[w] == j), stop=(last[w] == j))
                if last[w] == j:
                    nc.vector.tensor_add(out_sb[:, w, :], out_sb[:, w, :],
                                         ptile[w][:])
                    del ptile[w]
        assert not ptile


def _build_and_run(in_maps, adj_plan, hv_plan, hu_plan):
    global last_exec_wall
    nc = bacc.Bacc("TRN2", target_bir_lowering=False, debug=False, num_devices=NC)
    f32, i16 = mybir.dt.float32, mybir.dt.int16

    x0 = nc.dram_tensor("x0", [XROWS, D], f32, kind="ExternalInput")
    x0sh = nc.dram_tensor("x0sh", [128, NWIN, D], f32, kind="ExternalInput")
    item_aug = nc.dram_tensor("item_aug", [ITEM_AUG_ROWS, 128], f32, kind="ExternalInput")

    def edge_inputs(pfx, plan):
        ncall = max(len(c) for c in plan.calls)
        ixs = nc.dram_tensor(f"{pfx}_ix", [plan.nbuck, ncall, 128, CHUNK // 16], i16,
                             kind="ExternalInput")
        dvs = nc.dram_tensor(f"{pfx}_dv", [plan.nbuck, ncall, 128, CHUNK // 128], f32,
                             kind="ExternalInput")
        vls = nc.dram_tensor(f"{pfx}_vl", [plan.nbuck, ncall, 128, CHUNK // 128], f32,
                             kind="ExternalInput")
        return ixs, dvs, vls

    adj_io = edge_inputs("adj", adj_plan)
    hv_io = edge_inputs("hv", hv_plan)
    hu_io = edge_inputs("hu", hu_plan)

    out_u = nc.dram_tensor("out_u", [128, UWIN, D], f32, kind="ExternalOutput")
    out_i = nc.dram_tensor("out_i", [128, IPAD // 128, D], f32, kind="ExternalOutput")

    xf1 = nc.dram_tensor("xf1", [XROWS, D], f32, addr_space="Shared")
    xf2 = nc.dram_tensor("xf2", [XROWS, D], f32, addr_space="Shared")
    ebounce = nc.dram_tensor("ebounce", [NLOC, D], f32)
    bbounce = nc.dram_tensor("bbounce", [BPAD, 128], f32)
    bfull = nc.dram_tensor("bfull", [BROWS, 128], f32, addr_space="Shared")

    with tile.TileContext(nc) as tc:
        with tc.tile_pool(name="persist", bufs=1) as persist, \
             tc.tile_pool(name="ps", bufs=8, space="PSUM") as ps:
            iota0 = persist.tile([128, 128], f32)
            nc.gpsimd.iota(iota0[:], pattern=[[1, 128]], base=0, channel_multiplier=0,
                           allow_small_or_imprecise_dtypes=True)
            acc = persist.tile([128, NWIN, D], f32)
            nc.sync.dma_start(acc[:], x0sh[:])

            with tc.tile_pool(name="adjbig", bufs=1) as adjbig, \
                 tc.tile_pool(name="sb1", bufs=3) as sb:
                out_sb = adjbig.tile([128, NWIN, D], f32)
                for li, srct in enumerate([x0, xf1, xf2]):
                    nc.vector.memset(out_sb[:], 0.0)
                    _emit_spmm(nc, sb, ps, adj_plan, srct, XROWS, *adj_io,
                               out_sb, iota0, f"a{li}")
                    nc.vector.tensor_add(acc[:], acc[:], out_sb[:])
                    if li < 2:
                        nc.sync.dma_start(ebounce.rearrange("(n p) d -> p n d", p=128),
                                          out_sb[:])
                        nc.gpsimd.collective_compute(
                            "AllGather", mybir.AluOpType.bypass,
                            replica_groups=[list(range(NC))],
                            ins=[ebounce[:].opt()], outs=[[xf1, xf2][li][:].opt()])

            with tc.tile_pool(name="hubig", bufs=1) as hubig, \
                 tc.tile_pool(name="sb2", bufs=2) as sb:
                # ---- hv: items -> bicliques (aug col 64 = degree)
                bic = hubig.tile([128, BPAD // 128, 128], f32)
                nc.vector.memset(bic[:], 0.0)
                _emit_spmm(nc, sb, ps, hv_plan, item_aug, ITEM_AUG_ROWS, *hv_io,
                           bic, iota0, "hv")
                nbr = BPAD // 128
                deg = sb.tile([128, nbr], f32, tag="bdeg")
                rec = sb.tile([128, nbr], f32, tag="brec")
                nc.vector.scalar_tensor_tensor(deg[:], bic[:, :, 64], 0.0, bic[:, :, 64],
                                               mybir.AluOpType.is_equal,
                                               mybir.AluOpType.add)
                nc.vector.reciprocal(rec[:], deg[:])
                for r in range(nbr):
                    nc.vector.tensor_scalar_mul(bic[:, r, 0:64], bic[:, r, 0:64],
                                                rec[:, r:r + 1])
                nc.vector.memset(bic[:, :, 64:65], 1.0)
                nc.vector.memset(bic[:, :, 65:128], 0.0)
                nc.sync.dma_start(bbounce.rearrange("(n p) d -> p n d", p=128), bic[:])
                nc.gpsimd.collective_compute(
                    "AllGather", mybir.AluOpType.bypass,
                    replica_groups=[list(range(NC))],
                    ins=[bbounce[:].opt()], outs=[bfull[:].opt()])

                # ---- hu: bicliques -> users
                ul = hubig.tile([128, UWIN, 128], f32)
                nc.vector.memset(ul[:], 0.0)
                _emit_spmm(nc, sb, ps, hu_plan, bfull, BROWS, *hu_io,
                           ul, iota0, "hu")
                udeg = sb.tile([128, UWIN], f32, tag="udeg")
                urec = sb.tile([128, UWIN], f32, tag="urec")
                nc.vector.scalar_tensor_tensor(udeg[:], ul[:, :, 64], 0.0, ul[:, :, 64],
                                               mybir.AluOpType.is_equal,
                                               mybir.AluOpType.add)
                nc.vector.reciprocal(urec[:], udeg[:])
                nc.vector.tensor_scalar(acc[:], acc[:], 0.25, None,
                                        mybir.AluOpType.mult)
                for r in range(UWIN):
                    nc.vector.tensor_scalar(ul[:, r, 0:64], ul[:, r, 0:64],
                                            urec[:, r:r + 1], None,
                                            mybir.AluOpType.mult)
                    nc.vector.tensor_add(ul[:, r, 0:64], ul[:, r, 0:64],
                                         acc[:, r, :])
                nc.sync.dma_start(out_u[:], ul[:, :, 0:64])
                nc.sync.dma_start(out_i[:], acc[:, UWIN:NWIN, :])
    nc.compile()
    global last_exec_ns
    import os as _os
    trace = _os.environ.get("BASS_PROFILE", "0") == "1" and _install_ntff_hook()
    t0 = time.time()
    res = run_bass_kernel_spmd(nc, in_maps, list(range(NC)), trace=trace)
    last_exec_wall = time.time() - t0
    if trace:
        last_exec_ns = res.exec_time_ns
    return res


def _pack_inputs(stream, plan):
    ncall = max(len(c) for c in plan.calls)
    ix = np.zeros((plan.nbuck, ncall, 128, CHUNK // 16), np.int16)
    dv = np.zeros((plan.nbuck, ncall, 128, CHUNK // 128), np.float32)
    vl = np.zeros((plan.nbuck, ncall, 128, CHUNK // 128), np.float32)
    idx, dstr, val = stream
    boff = 0
    for b in range(plan.nbuck):
        for ci, (off, csz) in enumerate(plan.calls[b]):
            s = boff + off
            ix[b, ci, :, :csz // 16] = _wrap_idx(idx[s:s + csz])
            dv[b, ci, :, :csz // 128] = dstr[s:s + csz].reshape(-1, 128).T
            vl[b, ci, :, :csz // 128] = val[s:s + csz].reshape(-1, 128).T
        boff += plan.bslots[b]
    return ix, dv, vl


def kernel(user_emb, item_emb, adj_val, hv_val, hu_val,
           adj_row, adj_col, hv_row, hv_col, hu_row, hu_col):
    user_emb = np.asarray(user_emb, np.float32)
    item_emb = np.asarray(item_emb, np.float32)
    adj_val = np.asarray(adj_val, np.float32)
    hv_val = np.asarray(hv_val, np.float32)
    hu_val = np.asarray(hu_val, np.float32)
    adj_row = np.asarray(adj_row, np.int64)
    adj_col = np.asarray(adj_col, np.int64)
    hv_row = np.asarray(hv_row, np.int64)
    hv_col = np.asarray(hv_col, np.int64)
    hu_row = np.asarray(hu_row, np.int64)
    hu_col = np.asarray(hu_col, np.int64)

    x0 = np.zeros((XROWS, D), np.float32)
    allp = _perm_node(np.arange(U + I))
    x0[allp[:U]] = user_emb
    x0[allp[U:]] = item_emb

    item_aug = np.zeros((ITEM_AUG_ROWS, 128), np.float32)
    item_aug[:I, :64] = item_emb
    item_aug[:I, 64] = 1.0

    def core_of_node(g):
        g = np.asarray(g, np.int64)
        return np.where(g < U, np.minimum(g // UPC, NC - 1),
                        np.minimum((g - U) // IPC, NC - 1))

    def local_of_node(g):
        g = np.asarray(g, np.int64)
        k = core_of_node(g)
        return np.where(g < U, g - k * UPC, UPAD + (g - U) - k * IPC)

    adj_core = core_of_node(adj_row)
    adj_dl = local_of_node(adj_row)
    adj_cp = _perm_node(adj_col)
    hv_core = hv_row // BPC
    hv_dl = hv_row - hv_core * BPC
    hu_core = hu_row // UPC
    hu_dl = hu_row - hu_core * UPC
    hu_cp = hu_col // BPC * BPAD + hu_col % BPC

    adj_plan, adj_streams = _schedule(adj_core, adj_dl, adj_cp, adj_val, 5, NWIN, D)
    hv_plan, hv_streams = _schedule(hv_core, hv_dl, hv_col, hv_val, 2, BPAD // 128, 128)
    hu_plan, hu_streams = _schedule(hu_core, hu_dl, hu_cp, hu_val, 1, UWIN, 128)

    in_maps = []
    for k in range(NC):
        a = _pack_inputs(adj_streams[k], adj_plan)
        v = _pack_inputs(hv_streams[k], hv_plan)
        u = _pack_inputs(hu_streams[k], hu_plan)
        x0sh = x0[k * NLOC:(k + 1) * NLOC].reshape(NWIN, 128, D).transpose(1, 0, 2).copy()
        in_maps.append({
            "x0": x0, "x0sh": x0sh, "item_aug": item_aug,
            "adj_ix": a[0], "adj_dv": a[1], "adj_vl": a[2],
            "hv_ix": v[0], "hv_dv": v[1], "hv_vl": v[2],
            "hu_ix": u[0], "hu_dv": u[1], "hu_vl": u[2],
        })

    res = _build_and_run(in_maps, adj_plan, hv_plan, hu_plan)

    u_out = np.zeros((U, D), np.float32)
    i_out = np.zeros((I, D), np.float32)
    for k in range(NC):
        ou = res.results[k]["out_u"]
        oi = res.results[k]["out_i"]
        u_out[k * UPC:(k + 1) * UPC] = ou.transpose(1, 0, 2).reshape(UPAD, D)[:UPC]
        i_out[k * IPC:(k + 1) * IPC] = oi.transpose(1, 0, 2).reshape(IPAD, D)[:IPC]
    return u_out, i_out


# revision 9
# speedup vs baseline: 5722.0026x; 1.0015x over previous
import sys, time
if "/opt/trn_rl_repo" not in sys.path:
    sys.path.insert(0, "/opt/trn_rl_repo")
import numpy as np
import concourse.bass as bass
import concourse.bacc as bacc
from concourse import tile
from concourse.bass_utils import run_bass_kernel_spmd

mybir = bass.mybir

NC = 8
U, I, B = 100000, 50000, 20000
D = 64
UPC, IPC, BPC = U // NC, I // NC, B // NC          # 12500, 6250, 2500
UPAD, IPAD, BPAD = 12544, 6272, 2560               # multiples of 128
NLOC = UPAD + IPAD                                 # 18816 local rows per core
NWIN = NLOC // 128                                 # 147
UWIN = UPAD // 128                                 # 98
XROWS = NC * NLOC                                  # 150528 perm-table rows
BROWS = NC * BPAD                                  # 20480
ITEM_AUG_ROWS = 50176
BUCKET = 32768
CHUNK = 8192

last_exec_wall = None
last_exec_ns = None
last_res = None


def _install_ntff_hook():
    """Provide antenv.axon_hooks (absent in this image) so
    run_bass_kernel_spmd(trace=True) can NTFF-profile via libaxon_pjrt."""
    import types, ctypes, contextlib, importlib
    try:
        import antenv.axon_hooks  # noqa
        return True
    except ImportError:
        pass
    try:
        lib = ctypes.CDLL("/opt/axon/libaxon_pjrt.so")
        if not hasattr(lib, "axon_start_nrt_profile"):
            return False
        lib.axon_start_nrt_profile.argtypes = [ctypes.POINTER(ctypes.c_int64), ctypes.c_size_t]
        lib.axon_start_nrt_profile.restype = ctypes.c_int64
        lib.axon_stop_nrt_profile.argtypes = [ctypes.c_char_p]
        lib.axon_stop_nrt_profile.restype = ctypes.c_int64

        @contextlib.contextmanager
        def _hook(output_dir, device_ids):
            import jax
            jax.devices()
            if device_ids:
                ids = (ctypes.c_int64 * len(device_ids))(*device_ids)
                rc = lib.axon_start_nrt_profile(ids, len(device_ids))
            else:
                rc = lib.axon_start_nrt_profile(None, 0)
            try:
                yield
            finally:
                if rc == 0:
                    lib.axon_stop_nrt_profile(output_dir.encode())

        mod = types.ModuleType("antenv.axon_hooks")
        _state = {"hook": _hook}
        mod.get_axon_ntff_profile_hook = lambda: _state["hook"]
        mod.set_axon_ntff_profile_hook = lambda h: _state.update(hook=h)
        import antenv
        sys.modules["antenv.axon_hooks"] = mod
        antenv.axon_hooks = mod
        return True
    except Exception:
        return False


def _perm_node(g):
    g = np.asarray(g, np.int64)
    is_item = g >= U
    u = np.minimum(g, U - 1)
    ku = u // UPC
    pu = ku * NLOC + (u - ku * UPC)
    it = np.maximum(g - U, 0)
    ki = it // IPC
    pi = ki * NLOC + UPAD + (it - ki * IPC)
    return np.where(is_item, pi, pu)


def _wrap_idx(a):
    return np.tile(a.reshape(-1, 16).T, (8, 1)).astype(np.int16)


def _calls(slots):
    out, off = [], 0
    while off < slots:
        c = min(CHUNK, slots - off)
        out.append((off, c))
        off += c
    return out


class _Plan:
    """Static spmm schedule, identical across cores.
    T[b][w] = columns for (bucket, window)."""
    def __init__(self, nbuck, nwin, dout, T):
        self.nbuck, self.nwin, self.dout, self.T = nbuck, nwin, dout, T
        self.bslots = [int(T[b].sum()) * 128 for b in range(nbuck)]
        self.calls = [_calls(s) for s in self.bslots]
        self.colwin = [np.repeat(np.arange(nwin), T[b]) for b in range(nbuck)]


def _schedule(core, dl, cp, vv, nbuck, nwin, dout):
    """Build shared schedule + per-core slot streams."""
    key = (cp // BUCKET) * nwin + (dl // 128)
    counts = np.zeros((NC, nbuck * nwin), np.int64)
    for k in range(NC):
        counts[k] = np.bincount(key[core == k], minlength=nbuck * nwin)
    T = -(-counts.max(0) // 128)          # ceil(max/128), may be 0
    T = T.reshape(nbuck, nwin)
    plan = _Plan(nbuck, nwin, dout, T)
    gstart = np.concatenate([[0], np.cumsum(T.reshape(-1) * 128)])[:-1]

    streams = []
    for k in range(NC):
        m = core == k
        dlk, cpk, vvk = dl[m], cp[m], vv[m]
        keyk = (cpk // BUCKET) * nwin + (dlk // 128)
        o = np.argsort(keyk, kind="stable")
        dlk, cpk, vvk, keyk = dlk[o], cpk[o], vvk[o], keyk[o]
        ck = counts[k]
        ranks = np.arange(len(keyk)) - np.repeat(np.concatenate([[0], np.cumsum(ck)])[:-1], ck)
        slots = gstart[keyk] + ranks
        tot = int(T.sum()) * 128
        idx = np.zeros(tot, np.int16)
        dstr = np.zeros(tot, np.float32)
        val = np.zeros(tot, np.float32)
        idx[slots] = (cpk % BUCKET).astype(np.int16)
        dstr[slots] = (dlk % 128).astype(np.float32)
        val[slots] = vvk
        streams.append((idx, dstr, val))
    return plan, streams


def _emit_spmm(nc, sb, ps, plan, table_ap, table_rows, ix_dram, dv_dram, vl_dram,
               out_sb, iota0, tagp):
    for b in range(plan.nbuck):
        if plan.bslots[b] == 0:
            continue
        base_rows = b * BUCKET
        nrows = min(BUCKET, table_rows - base_rows)
        colwin = plan.colwin[b]
        ncol_total = plan.bslots[b] // 128
        first = {}
        last = {}
        for j in range(ncol_total):
            w = int(colwin[j])
            first.setdefault(w, j)
            last[w] = j
        ptile = {}
        for ci, (off, csz) in enumerate(plan.calls[b]):
            ccols = csz // 128
            ix = sb.tile([128, CHUNK // 16], mybir.dt.int16, tag="ix")
            nc.sync.dma_start(ix[:, :csz // 16], ix_dram[b, ci, :, :csz // 16])
            gat = sb.tile([128, CHUNK // 128, plan.dout], mybir.dt.float32, tag="g")
            nc.gpsimd.dma_gather(gat[:, :ccols, :],
                                 table_ap[base_rows:base_rows + nrows, :],
                                 ix[:, :csz // 16], csz, csz, plan.dout,
                                 single_packet=False)
            dv = sb.tile([128, CHUNK // 128], mybir.dt.float32, tag="d")
            vl = sb.tile([128, CHUNK // 128], mybir.dt.float32, tag="v")
            nc.sync.dma_start(dv[:, :ccols], dv_dram[b, ci, :, :ccols])
            nc.sync.dma_start(vl[:, :ccols], vl_dram[b, ci, :, :ccols])
            for lc in range(ccols):
                j = off // 128 + lc
                w = int(colwin[j])
                S = sb.tile([128, 128], mybir.dt.float32, tag="S")
                nc.vector.tensor_scalar(S[:], iota0[:], dv[:, lc:lc + 1],
                                        vl[:, lc:lc + 1],
                                        mybir.AluOpType.is_equal,
                                        mybir.AluOpType.mult)
                if w not in ptile:
                    ptile[w] = ps.tile([128, plan.dout], mybir.dt.float32,
                                       tag="p", name=f"pw{tagp}_{w}")
                nc.tensor.matmul(ptile[w][:], lhsT=S[:], rhs=gat[:, lc, :],
                                 start=(first[w] == j), stop=(last[w] == j))
                if last[w] == j:
                    nc.vector.tensor_add(out_sb[:, w, :], out_sb[:, w, :],
                                         ptile[w][:])
                    del ptile[w]
        assert not ptile


def _build_and_run(in_maps, adj_plan, hv_plan, hu_plan):
    global last_exec_wall
    nc = bacc.Bacc("TRN2", target_bir_lowering=False, debug=False, num_devices=NC)
    f32, i16 = mybir.dt.float32, mybir.dt.int16

    x0 = nc.dram_tensor("x0", [XROWS, D], f32, kind="ExternalInput")
    x0sh = nc.dram_tensor("x0sh", [128, NWIN, D], f32, kind="ExternalInput")
    item_aug = nc.dram_tensor("item_aug", [ITEM_AUG_ROWS, 128], f32, kind="ExternalInput")

    def edge_inputs(pfx, plan):
        ncall = max(len(c) for c in plan.calls)
        ixs = nc.dram_tensor(f"{pfx}_ix", [plan.nbuck, ncall, 128, CHUNK // 16], i16,
                             kind="ExternalInput")
        dvs = nc.dram_tensor(f"{pfx}_dv", [plan.nbuck, ncall, 128, CHUNK // 128], f32,
                             kind="ExternalInput")
        vls = nc.dram_tensor(f"{pfx}_vl", [plan.nbuck, ncall, 128, CHUNK // 128], f32,
                             kind="ExternalInput")
        return ixs, dvs, vls

    adj_io = edge_inputs("adj", adj_plan)
    hv_io = edge_inputs("hv", hv_plan)
    hu_io = edge_inputs("hu", hu_plan)

    out_u = nc.dram_tensor("out_u", [128, UWIN, D], f32, kind="ExternalOutput")
    out_i = nc.dram_tensor("out_i", [128, IPAD // 128, D], f32, kind="ExternalOutput")

    xf1 = nc.dram_tensor("xf1", [XROWS, D], f32, addr_space="Shared")
    xf2 = nc.dram_tensor("xf2", [XROWS, D], f32, addr_space="Shared")
    ebounce = nc.dram_tensor("ebounce", [NLOC, D], f32)
    bbounce = nc.dram_tensor("bbounce", [BPAD, 128], f32)
    bfull = nc.dram_tensor("bfull", [BROWS, 128], f32, addr_space="Shared")

    with tile.TileContext(nc) as tc:
        with tc.tile_pool(name="persist", bufs=1) as persist, \
             tc.tile_pool(name="ps", bufs=8, space="PSUM") as ps:
            iota0 = persist.tile([128, 128], f32)
            nc.gpsimd.iota(iota0[:], pattern=[[1, 128]], base=0, channel_multiplier=0,
                           allow_small_or_imprecise_dtypes=True)
            acc = persist.tile([128, NWIN, D], f32)
            nc.sync.dma_start(acc[:], x0sh[:])

            with tc.tile_pool(name="adjbig", bufs=1) as adjbig, \
                 tc.tile_pool(name="sb1", bufs=4) as sb:
                out_sb = adjbig.tile([128, NWIN, D], f32)
                for li, srct in enumerate([x0, xf1, xf2]):
                    nc.vector.memset(out_sb[:], 0.0)
                    _emit_spmm(nc, sb, ps, adj_plan, srct, XROWS, *adj_io,
                               out_sb, iota0, f"a{li}")
                    nc.vector.tensor_add(acc[:], acc[:], out_sb[:])
                    if li < 2:
                        nc.sync.dma_start(ebounce.rearrange("(n p) d -> p n d", p=128),
                                          out_sb[:])
                        nc.gpsimd.collective_compute(
                            "AllGather", mybir.AluOpType.bypass,
                            replica_groups=[list(range(NC))],
                            ins=[ebounce[:].opt()], outs=[[xf1, xf2][li][:].opt()])

            with tc.tile_pool(name="hubig", bufs=1) as hubig, \
                 tc.tile_pool(name="sb2", bufs=2) as sb:
                # ---- hv: items -> bicliques (aug col 64 = degree)
                bic = hubig.tile([128, BPAD // 128, 128], f32)
                nc.vector.memset(bic[:], 0.0)
                _emit_spmm(nc, sb, ps, hv_plan, item_aug, ITEM_AUG_ROWS, *hv_io,
                           bic, iota0, "hv")
                nbr = BPAD // 128
                deg = sb.tile([128, nbr], f32, tag="bdeg")
                rec = sb.tile([128, nbr], f32, tag="brec")
                nc.vector.scalar_tensor_tensor(deg[:], bic[:, :, 64], 0.0, bic[:, :, 64],
                                               mybir.AluOpType.is_equal,
                                               mybir.AluOpType.add)
                nc.vector.reciprocal(rec[:], deg[:])
                for r in range(nbr):
                    nc.vector.tensor_scalar_mul(bic[:, r, 0:64], bic[:, r, 0:64],
                                                rec[:, r:r + 1])
                nc.vector.memset(bic[:, :, 64:65], 1.0)
                nc.vector.memset(bic[:, :, 65:128], 0.0)
                nc.sync.dma_start(bbounce.rearrange("(n p) d -> p n d", p=128), bic[:])
                nc.gpsimd.collective_compute(
                    "AllGather", mybir.AluOpType.bypass,
                    replica_groups=[list(range(NC))],
                    ins=[bbounce[:].opt()], outs=[bfull[:].opt()])

                # ---- hu: bicliques -> users
                ul = hubig.tile([128, UWIN, 128], f32)
                nc.vector.memset(ul[:], 0.0)
                _emit_spmm(nc, sb, ps, hu_plan, bfull, BROWS, *hu_io,
                           ul, iota0, "hu")
                udeg = sb.tile([128, UWIN], f32, tag="udeg")
                urec = sb.tile([128, UWIN], f32, tag="urec")
                nc.vector.scalar_tensor_tensor(udeg[:], ul[:, :, 64], 0.0, ul[:, :, 64],
                                               mybir.AluOpType.is_equal,
                                               mybir.AluOpType.add)
                nc.vector.reciprocal(urec[:], udeg[:])
                nc.vector.tensor_scalar(acc[:], acc[:], 0.25, None,
                                        mybir.AluOpType.mult)
                for r in range(UWIN):
                    nc.vector.tensor_scalar(ul[:, r, 0:64], ul[:, r, 0:64],
                                            urec[:, r:r + 1], None,
                                            mybir.AluOpType.mult)
                    nc.vector.tensor_add(ul[:, r, 0:64], ul[:, r, 0:64],
                                         acc[:, r, :])
                nc.sync.dma_start(out_u[:], ul[:, :, 0:64])
                nc.sync.dma_start(out_i[:], acc[:, UWIN:NWIN, :])
    nc.compile()
    global last_exec_ns, last_res
    import os as _os
    trace = _os.environ.get("BASS_PROFILE", "0") == "1" and _install_ntff_hook()
    t0 = time.time()
    res = run_bass_kernel_spmd(nc, in_maps, list(range(NC)), trace=trace)
    last_exec_wall = time.time() - t0
    if trace:
        last_exec_ns = res.exec_time_ns
        last_res = res
    return res


def _pack_inputs(stream, plan):
    ncall = max(len(c) for c in plan.calls)
    ix = np.zeros((plan.nbuck, ncall, 128, CHUNK // 16), np.int16)
    dv = np.zeros((plan.nbuck, ncall, 128, CHUNK // 128), np.float32)
    vl = np.zeros((plan.nbuck, ncall, 128, CHUNK // 128), np.float32)
    idx, dstr, val = stream
    boff = 0
    for b in range(plan.nbuck):
        for ci, (off, csz) in enumerate(plan.calls[b]):
            s = boff + off
            ix[b, ci, :, :csz // 16] = _wrap_idx(idx[s:s + csz])
            dv[b, ci, :, :csz // 128] = dstr[s:s + csz].reshape(-1, 128).T
            vl[b, ci, :, :csz // 128] = val[s:s + csz].reshape(-1, 128).T
        boff += plan.bslots[b]
    return ix, dv, vl


def kernel(user_emb, item_emb, adj_val, hv_val, hu_val,
           adj_row, adj_col, hv_row, hv_col, hu_row, hu_col):
    user_emb = np.asarray(user_emb, np.float32)
    item_emb = np.asarray(item_emb, np.float32)
    adj_val = np.asarray(adj_val, np.float32)
    hv_val = np.asarray(hv_val, np.float32)
    hu_val = np.asarray(hu_val, np.float32)
    adj_row = np.asarray(adj_row, np.int64)
    adj_col = np.asarray(adj_col, np.int64)
    hv_row = np.asarray(hv_row, np.int64)
    hv_col = np.asarray(hv_col, np.int64)
    hu_row = np.asarray(hu_row, np.int64)
    hu_col = np.asarray(hu_col, np.int64)

    x0 = np.zeros((XROWS, D), np.float32)
    allp = _perm_node(np.arange(U + I))
    x0[allp[:U]] = user_emb
    x0[allp[U:]] = item_emb

    item_aug = np.zeros((ITEM_AUG_ROWS, 128), np.float32)
    item_aug[:I, :64] = item_emb
    item_aug[:I, 64] = 1.0

    def core_of_node(g):
        g = np.asarray(g, np.int64)
        return np.where(g < U, np.minimum(g // UPC, NC - 1),
                        np.minimum((g - U) // IPC, NC - 1))

    def local_of_node(g):
        g = np.asarray(g, np.int64)
        k = core_of_node(g)
        return np.where(g < U, g - k * UPC, UPAD + (g - U) - k * IPC)

    adj_core = core_of_node(adj_row)
    adj_dl = local_of_node(adj_row)
    adj_cp = _perm_node(adj_col)
    hv_core = hv_row // BPC
    hv_dl = hv_row - hv_core * BPC
    hu_core = hu_row // UPC
    hu_dl = hu_row - hu_core * UPC
    hu_cp = hu_col // BPC * BPAD + hu_col % BPC

    adj_plan, adj_streams = _schedule(adj_core, adj_dl, adj_cp, adj_val, 5, NWIN, D)
    hv_plan, hv_streams = _schedule(hv_core, hv_dl, hv_col, hv_val, 2, BPAD // 128, 128)
    hu_plan, hu_streams = _schedule(hu_core, hu_dl, hu_cp, hu_val, 1, UWIN, 128)

    in_maps = []
    for k in range(NC):
        a = _pack_inputs(adj_streams[k], adj_plan)
        v = _pack_inputs(hv_streams[k], hv_plan)
        u = _pack_inputs(hu_streams[k], hu_plan)
        x0sh = x0[k * NLOC:(k + 1) * NLOC].reshape(NWIN, 128, D).transpose(1, 0, 2).copy()
        in_maps.append({
            "x0": x0, "x0sh": x0sh, "item_aug": item_aug,
            "adj_ix": a[0], "adj_dv": a[1], "adj_vl": a[2],
            "hv_ix": v[0], "hv_dv": v[1], "hv_vl": v[2],
            "hu_ix": u[0], "hu_dv": u[1], "hu_vl": u[2],
        })

    res = _build_and_run(in_maps, adj_plan, hv_plan, hu_plan)

    u_out = np.zeros((U, D), np.float32)
    i_out = np.zeros((I, D), np.float32)
    for k in range(NC):
        ou = res.results[k]["out_u"]
        oi = res.results[k]["out_i"]
        u_out[k * UPC:(k + 1) * UPC] = ou.transpose(1, 0, 2).reshape(UPAD, D)[:UPC]
        i_out[k * IPC:(k + 1) * IPC] = oi.transpose(1, 0, 2).reshape(IPAD, D)[:IPC]
    return u_out, i_out


# revision 11
# speedup vs baseline: 5979.1764x; 1.0449x over previous
import sys, time
if "/opt/trn_rl_repo" not in sys.path:
    sys.path.insert(0, "/opt/trn_rl_repo")
import numpy as np
import concourse.bass as bass
import concourse.bacc as bacc
from concourse import tile
from concourse.bass_utils import run_bass_kernel_spmd

mybir = bass.mybir

NC = 8
U, I, B = 100000, 50000, 20000
D = 64
UPC, IPC, BPC = U // NC, I // NC, B // NC          # 12500, 6250, 2500
UPAD, IPAD, BPAD = 12544, 6272, 2560               # multiples of 128
NLOC = UPAD + IPAD                                 # 18816 local rows per core
NWIN = NLOC // 128                                 # 147
UWIN = UPAD // 128                                 # 98
XROWS = NC * NLOC                                  # 150528 perm-table rows
BROWS = NC * BPAD                                  # 20480
ITEM_AUG_ROWS = 50176
BUCKET = 32768
CHUNK = 8192

last_exec_wall = None
last_exec_ns = None
last_res = None


def _install_ntff_hook():
    """Provide antenv.axon_hooks (absent in this image) so
    run_bass_kernel_spmd(trace=True) can NTFF-profile via libaxon_pjrt."""
    import types, ctypes, contextlib, importlib
    try:
        import antenv.axon_hooks  # noqa
        return True
    except ImportError:
        pass
    try:
        lib = ctypes.CDLL("/opt/axon/libaxon_pjrt.so")
        if not hasattr(lib, "axon_start_nrt_profile"):
            return False
        lib.axon_start_nrt_profile.argtypes = [ctypes.POINTER(ctypes.c_int64), ctypes.c_size_t]
        lib.axon_start_nrt_profile.restype = ctypes.c_int64
        lib.axon_stop_nrt_profile.argtypes = [ctypes.c_char_p]
        lib.axon_stop_nrt_profile.restype = ctypes.c_int64

        @contextlib.contextmanager
        def _hook(output_dir, device_ids):
            import jax
            jax.devices()
            if device_ids:
                ids = (ctypes.c_int64 * len(device_ids))(*device_ids)
                rc = lib.axon_start_nrt_profile(ids, len(device_ids))
            else:
                rc = lib.axon_start_nrt_profile(None, 0)
            try:
                yield
            finally:
                if rc == 0:
                    lib.axon_stop_nrt_profile(output_dir.encode())

        mod = types.ModuleType("antenv.axon_hooks")
        _state = {"hook": _hook}
        mod.get_axon_ntff_profile_hook = lambda: _state["hook"]
        mod.set_axon_ntff_profile_hook = lambda h: _state.update(hook=h)
        import antenv
        sys.modules["antenv.axon_hooks"] = mod
        antenv.axon_hooks = mod
        return True
    except Exception:
        return False


def _perm_node(g):
    g = np.asarray(g, np.int64)
    is_item = g >= U
    u = np.minimum(g, U - 1)
    ku = u // UPC
    pu = ku * NLOC + (u - ku * UPC)
    it = np.maximum(g - U, 0)
    ki = it // IPC
    pi = ki * NLOC + UPAD + (it - ki * IPC)
    return np.where(is_item, pi, pu)


def _wrap_idx(a):
    return np.tile(a.reshape(-1, 16).T, (8, 1)).astype(np.int16)


def _calls(slots):
    out, off = [], 0
    while off < slots:
        c = min(CHUNK, slots - off)
        out.append((off, c))
        off += c
    return out


class _Plan:
    """Static spmm schedule, identical across cores.
    T[b][w] = columns for (bucket, window)."""
    def __init__(self, nbuck, nwin, dout, T):
        self.nbuck, self.nwin, self.dout, self.T = nbuck, nwin, dout, T
        self.bslots = [int(T[b].sum()) * 128 for b in range(nbuck)]
        self.calls = [_calls(s) for s in self.bslots]
        self.colwin = [np.repeat(np.arange(nwin), T[b]) for b in range(nbuck)]


def _schedule(core, dl, cp, vv, nbuck, nwin, dout):
    """Build shared schedule + per-core slot streams."""
    key = (cp // BUCKET) * nwin + (dl // 128)
    counts = np.zeros((NC, nbuck * nwin), np.int64)
    for k in range(NC):
        counts[k] = np.bincount(key[core == k], minlength=nbuck * nwin)
    T = -(-counts.max(0) // 128)          # ceil(max/128), may be 0
    T = T.reshape(nbuck, nwin)
    plan = _Plan(nbuck, nwin, dout, T)
    gstart = np.concatenate([[0], np.cumsum(T.reshape(-1) * 128)])[:-1]

    streams = []
    for k in range(NC):
        m = core == k
        dlk, cpk, vvk = dl[m], cp[m], vv[m]
        keyk = (cpk // BUCKET) * nwin + (dlk // 128)
        o = np.argsort(keyk, kind="stable")
        dlk, cpk, vvk, keyk = dlk[o], cpk[o], vvk[o], keyk[o]
        ck = counts[k]
        ranks = np.arange(len(keyk)) - np.repeat(np.concatenate([[0], np.cumsum(ck)])[:-1], ck)
        slots = gstart[keyk] + ranks
        tot = int(T.sum()) * 128
        idx = np.zeros(tot, np.int16)
        dstr = np.zeros(tot, np.float32)
        val = np.zeros(tot, np.float32)
        idx[slots] = (cpk % BUCKET).astype(np.int16)
        dstr[slots] = (dlk % 128).astype(np.float32)
        val[slots] = vvk
        streams.append((idx, dstr, val))
    return plan, streams


def _emit_spmm(nc, sb, ps, plan, table_ap, table_rows, ix_dram, dv_dram, vl_dram,
               out_sb, iota0, tagp):
    for b in range(plan.nbuck):
        if plan.bslots[b] == 0:
            continue
        base_rows = b * BUCKET
        nrows = min(BUCKET, table_rows - base_rows)
        colwin = plan.colwin[b]
        ncol_total = plan.bslots[b] // 128
        first = {}
        last = {}
        for j in range(ncol_total):
            w = int(colwin[j])
            first.setdefault(w, j)
            last[w] = j
        ptile = {}
        for ci, (off, csz) in enumerate(plan.calls[b]):
            ccols = csz // 128
            ix = sb.tile([128, CHUNK // 16], mybir.dt.int16, tag="ix")
            nc.sync.dma_start(ix[:, :csz // 16], ix_dram[b, ci, :, :csz // 16])
            gat = sb.tile([128, CHUNK // 128, plan.dout], mybir.dt.float32, tag="g")
            nc.gpsimd.dma_gather(gat[:, :ccols, :],
                                 table_ap[base_rows:base_rows + nrows, :],
                                 ix[:, :csz // 16], csz, csz, plan.dout,
                                 single_packet=False)
            dv = sb.tile([128, CHUNK // 128], mybir.dt.float32, tag="d")
            vl = sb.tile([128, CHUNK // 128], mybir.dt.float32, tag="v")
            nc.sync.dma_start(dv[:, :ccols], dv_dram[b, ci, :, :ccols])
            nc.sync.dma_start(vl[:, :ccols], vl_dram[b, ci, :, :ccols])
            for lc in range(ccols):
                j = off // 128 + lc
                w = int(colwin[j])
                S = sb.tile([128, 128], mybir.dt.float32, tag="S")
                nc.vector.tensor_scalar(S[:], iota0[:], dv[:, lc:lc + 1],
                                        vl[:, lc:lc + 1],
                                        mybir.AluOpType.is_equal,
                                        mybir.AluOpType.mult)
                if w not in ptile:
                    ptile[w] = ps.tile([128, plan.dout], mybir.dt.float32,
                                       tag="p", name=f"pw{tagp}_{w}")
                nc.tensor.matmul(ptile[w][:], lhsT=S[:], rhs=gat[:, lc, :],
                                 start=(first[w] == j), stop=(last[w] == j))
                if last[w] == j:
                    nc.vector.tensor_add(out_sb[:, w, :], out_sb[:, w, :],
                                         ptile[w][:])
                    del ptile[w]
        assert not ptile


def _build_and_run(in_maps, adj_plan, hv_plan, hu_plan):
    global last_exec_wall
    nc = bacc.Bacc("TRN2", target_bir_lowering=False, debug=False, num_devices=NC)
    f32, i16 = mybir.dt.float32, mybir.dt.int16

    x0 = nc.dram_tensor("x0", [XROWS, D], f32, kind="ExternalInput")
    x0sh = nc.dram_tensor("x0sh", [128, NWIN, D], f32, kind="ExternalInput")
    item_aug = nc.dram_tensor("item_aug", [ITEM_AUG_ROWS, 128], f32, kind="ExternalInput")

    def edge_inputs(pfx, plan):
        ncall = max(len(c) for c in plan.calls)
        ixs = nc.dram_tensor(f"{pfx}_ix", [plan.nbuck, ncall, 128, CHUNK // 16], i16,
                             kind="ExternalInput")
        dvs = nc.dram_tensor(f"{pfx}_dv", [plan.nbuck, ncall, 128, CHUNK // 128], f32,
                             kind="ExternalInput")
        vls = nc.dram_tensor(f"{pfx}_vl", [plan.nbuck, ncall, 128, CHUNK // 128], f32,
                             kind="ExternalInput")
        return ixs, dvs, vls

    adj_io = edge_inputs("adj", adj_plan)
    hv_io = edge_inputs("hv", hv_plan)
    hu_io = edge_inputs("hu", hu_plan)

    out_u = nc.dram_tensor("out_u", [128, UWIN, D], f32, kind="ExternalOutput")
    out_i = nc.dram_tensor("out_i", [128, IPAD // 128, D], f32, kind="ExternalOutput")

    xf1 = nc.dram_tensor("xf1", [XROWS, D], f32, addr_space="Shared")
    xf2 = nc.dram_tensor("xf2", [XROWS, D], f32, addr_space="Shared")
    ebounce = nc.dram_tensor("ebounce", [NLOC, D], f32)
    bbounce = nc.dram_tensor("bbounce", [BPAD, 128], f32)
    bfull = nc.dram_tensor("bfull", [BROWS, 128], f32, addr_space="Shared")

    with tile.TileContext(nc) as tc:
        with tc.tile_pool(name="persist", bufs=1) as persist, \
             tc.tile_pool(name="ps", bufs=8, space="PSUM") as ps:
            iota0 = persist.tile([128, 128], f32)
            nc.gpsimd.iota(iota0[:], pattern=[[1, 128]], base=0, channel_multiplier=0,
                           allow_small_or_imprecise_dtypes=True)
            acc = persist.tile([128, NWIN, D], f32)
            nc.sync.dma_start(acc[:], x0sh[:])

            with tc.tile_pool(name="adjbig", bufs=1) as adjbig, \
                 tc.tile_pool(name="sb1", bufs=4) as sb:
                out_sb = adjbig.tile([128, NWIN, D], f32)
                for li, srct in enumerate([x0, xf1, xf2]):
                    nc.vector.memset(out_sb[:], 0.0)
                    _emit_spmm(nc, sb, ps, adj_plan, srct, XROWS, *adj_io,
                               out_sb, iota0, f"a{li}")
                    nc.vector.tensor_add(acc[:], acc[:], out_sb[:])
                    if li < 2:
                        nc.sync.dma_start(ebounce.rearrange("(n p) d -> p n d", p=128),
                                          out_sb[:])
                        nc.gpsimd.collective_compute(
                            "AllGather", mybir.AluOpType.bypass,
                            replica_groups=[list(range(NC))],
                            ins=[ebounce[:].opt()], outs=[[xf1, xf2][li][:].opt()])

            with tc.tile_pool(name="hubig", bufs=1) as hubig, \
                 tc.tile_pool(name="sb2", bufs=3) as sb:
                # ---- hv: items -> bicliques (aug col 64 = degree)
                bic = hubig.tile([128, BPAD // 128, 128], f32)
                nc.vector.memset(bic[:], 0.0)
                _emit_spmm(nc, sb, ps, hv_plan, item_aug, ITEM_AUG_ROWS, *hv_io,
                           bic, iota0, "hv")
                nbr = BPAD // 128
                deg = sb.tile([128, nbr], f32, tag="bdeg")
                rec = sb.tile([128, nbr], f32, tag="brec")
                nc.vector.scalar_tensor_tensor(deg[:], bic[:, :, 64], 0.0, bic[:, :, 64],
                                               mybir.AluOpType.is_equal,
                                               mybir.AluOpType.add)
                nc.vector.reciprocal(rec[:], deg[:])
                for r in range(nbr):
                    nc.vector.tensor_scalar_mul(bic[:, r, 0:64], bic[:, r, 0:64],
                                                rec[:, r:r + 1])
                nc.vector.memset(bic[:, :, 64:65], 1.0)
                nc.vector.memset(bic[:, :, 65:128], 0.0)
                nc.sync.dma_start(bbounce.rearrange("(n p) d -> p n d", p=128), bic[:])
                nc.gpsimd.collective_compute(
                    "AllGather", mybir.AluOpType.bypass,
                    replica_groups=[list(range(NC))],
                    ins=[bbounce[:].opt()], outs=[bfull[:].opt()])

                # ---- hu: bicliques -> users
                ul = hubig.tile([128, UWIN, 128], f32)
                nc.vector.memset(ul[:], 0.0)
                _emit_spmm(nc, sb, ps, hu_plan, bfull, BROWS, *hu_io,
                           ul, iota0, "hu")
                udeg = sb.tile([128, UWIN], f32, tag="udeg")
                urec = sb.tile([128, UWIN], f32, tag="urec")
                nc.vector.scalar_tensor_tensor(udeg[:], ul[:, :, 64], 0.0, ul[:, :, 64],
                                               mybir.AluOpType.is_equal,
                                               mybir.AluOpType.add)
                nc.vector.reciprocal(urec[:], udeg[:])
                nc.vector.tensor_scalar(acc[:], acc[:], 0.25, None,
                                        mybir.AluOpType.mult)
                for r in range(UWIN):
                    nc.vector.tensor_scalar(ul[:, r, 0:64], ul[:, r, 0:64],
                                            urec[:, r:r + 1], None,
                                            mybir.AluOpType.mult)
                    nc.vector.tensor_add(ul[:, r, 0:64], ul[:, r, 0:64],
                                         acc[:, r, :])
                nc.sync.dma_start(out_u[:], ul[:, :, 0:64])
                nc.sync.dma_start(out_i[:], acc[:, UWIN:NWIN, :])
    nc.compile()
    global last_exec_ns, last_res
    import os as _os
    trace = _os.environ.get("BASS_PROFILE", "0") == "1" and _install_ntff_hook()
    t0 = time.time()
    res = run_bass_kernel_spmd(nc, in_maps, list(range(NC)), trace=trace)
    last_exec_wall = time.time() - t0
    if trace:
        last_exec_ns = res.exec_time_ns
        last_res = res
    return res


def _pack_inputs(stream, plan):
    ncall = max(len(c) for c in plan.calls)
    ix = np.zeros((plan.nbuck, ncall, 128, CHUNK // 16), np.int16)
    dv = np.zeros((plan.nbuck, ncall, 128, CHUNK // 128), np.float32)
    vl = np.zeros((plan.nbuck, ncall, 128, CHUNK // 128), np.float32)
    idx, dstr, val = stream
    boff = 0
    for b in range(plan.nbuck):
        for ci, (off, csz) in enumerate(plan.calls[b]):
            s = boff + off
            ix[b, ci, :, :csz // 16] = _wrap_idx(idx[s:s + csz])
            dv[b, ci, :, :csz // 128] = dstr[s:s + csz].reshape(-1, 128).T
            vl[b, ci, :, :csz // 128] = val[s:s + csz].reshape(-1, 128).T
        boff += plan.bslots[b]
    return ix, dv, vl


def kernel(user_emb, item_emb, adj_val, hv_val, hu_val,
           adj_row, adj_col, hv_row, hv_col, hu_row, hu_col):
    user_emb = np.asarray(user_emb, np.float32)
    item_emb = np.asarray(item_emb, np.float32)
    adj_val = np.asarray(adj_val, np.float32)
    hv_val = np.asarray(hv_val, np.float32)
    hu_val = np.asarray(hu_val, np.float32)
    adj_row = np.asarray(adj_row, np.int64)
    adj_col = np.asarray(adj_col, np.int64)
    hv_row = np.asarray(hv_row, np.int64)
    hv_col = np.asarray(hv_col, np.int64)
    hu_row = np.asarray(hu_row, np.int64)
    hu_col = np.asarray(hu_col, np.int64)

    x0 = np.zeros((XROWS, D), np.float32)
    allp = _perm_node(np.arange(U + I))
    x0[allp[:U]] = user_emb
    x0[allp[U:]] = item_emb

    item_aug = np.zeros((ITEM_AUG_ROWS, 128), np.float32)
    item_aug[:I, :64] = item_emb
    item_aug[:I, 64] = 1.0

    def core_of_node(g):
        g = np.asarray(g, np.int64)
        return np.where(g < U, np.minimum(g // UPC, NC - 1),
                        np.minimum((g - U) // IPC, NC - 1))

    def local_of_node(g):
        g = np.asarray(g, np.int64)
        k = core_of_node(g)
        return np.where(g < U, g - k * UPC, UPAD + (g - U) - k * IPC)

    adj_core = core_of_node(adj_row)
    adj_dl = local_of_node(adj_row)
    adj_cp = _perm_node(adj_col)
    hv_core = hv_row // BPC
    hv_dl = hv_row - hv_core * BPC
    hu_core = hu_row // UPC
    hu_dl = hu_row - hu_core * UPC
    hu_cp = hu_col // BPC * BPAD + hu_col % BPC

    adj_plan, adj_streams = _schedule(adj_core, adj_dl, adj_cp, adj_val, 5, NWIN, D)
    hv_plan, hv_streams = _schedule(hv_core, hv_dl, hv_col, hv_val, 2, BPAD // 128, 128)
    hu_plan, hu_streams = _schedule(hu_core, hu_dl, hu_cp, hu_val, 1, UWIN, 128)

    in_maps = []
    for k in range(NC):
        a = _pack_inputs(adj_streams[k], adj_plan)
        v = _pack_inputs(hv_streams[k], hv_plan)
        u = _pack_inputs(hu_streams[k], hu_plan)
        x0sh = x0[k * NLOC:(k + 1) * NLOC].reshape(NWIN, 128, D).transpose(1, 0, 2).copy()
        in_maps.append({
            "x0": x0, "x0sh": x0sh, "item_aug": item_aug,
            "adj_ix": a[0], "adj_dv": a[1], "adj_vl": a[2],
            "hv_ix": v[0], "hv_dv": v[1], "hv_vl": v[2],
            "hu_ix": u[0], "hu_dv": u[1], "hu_vl": u[2],
        })

    res = _build_and_run(in_maps, adj_plan, hv_plan, hu_plan)

    u_out = np.zeros((U, D), np.float32)
    i_out = np.zeros((I, D), np.float32)
    for k in range(NC):
        ou = res.results[k]["out_u"]
        oi = res.results[k]["out_i"]
        u_out[k * UPC:(k + 1) * UPC] = ou.transpose(1, 0, 2).reshape(UPAD, D)[:UPC]
        i_out[k * IPC:(k + 1) * IPC] = oi.transpose(1, 0, 2).reshape(IPAD, D)[:IPC]
    return u_out, i_out


# revision 12
# speedup vs baseline: 6404.3960x; 1.0711x over previous
import sys, time
if "/opt/trn_rl_repo" not in sys.path:
    sys.path.insert(0, "/opt/trn_rl_repo")
import numpy as np
import concourse.bass as bass
import concourse.bacc as bacc
from concourse import tile
from concourse.bass_utils import run_bass_kernel_spmd

mybir = bass.mybir

NC = 8
U, I, B = 100000, 50000, 20000
D = 64
UPC, IPC, BPC = U // NC, I // NC, B // NC          # 12500, 6250, 2500
UPAD, IPAD, BPAD = 12544, 6272, 2560               # multiples of 128
NLOC = UPAD + IPAD                                 # 18816 local rows per core
NWIN = NLOC // 128                                 # 147
UWIN = UPAD // 128                                 # 98
XROWS = NC * NLOC                                  # 150528 perm-table rows
BROWS = NC * BPAD                                  # 20480
ITEM_AUG_ROWS = 50176
BUCKET = 32768
CHUNK = 8192

last_exec_wall = None
last_exec_ns = None
last_res = None


def _install_ntff_hook():
    """Provide antenv.axon_hooks (absent in this image) so
    run_bass_kernel_spmd(trace=True) can NTFF-profile via libaxon_pjrt."""
    import types, ctypes, contextlib, importlib
    try:
        import antenv.axon_hooks  # noqa
        return True
    except ImportError:
        pass
    try:
        lib = ctypes.CDLL("/opt/axon/libaxon_pjrt.so")
        if not hasattr(lib, "axon_start_nrt_profile"):
            return False
        lib.axon_start_nrt_profile.argtypes = [ctypes.POINTER(ctypes.c_int64), ctypes.c_size_t]
        lib.axon_start_nrt_profile.restype = ctypes.c_int64
        lib.axon_stop_nrt_profile.argtypes = [ctypes.c_char_p]
        lib.axon_stop_nrt_profile.restype = ctypes.c_int64

        @contextlib.contextmanager
        def _hook(output_dir, device_ids):
            import jax
            jax.devices()
            if device_ids:
                ids = (ctypes.c_int64 * len(device_ids))(*device_ids)
                rc = lib.axon_start_nrt_profile(ids, len(device_ids))
            else:
                rc = lib.axon_start_nrt_profile(None, 0)
            try:
                yield
            finally:
                if rc == 0:
                    lib.axon_stop_nrt_profile(output_dir.encode())

        mod = types.ModuleType("antenv.axon_hooks")
        _state = {"hook": _hook}
        mod.get_axon_ntff_profile_hook = lambda: _state["hook"]
        mod.set_axon_ntff_profile_hook = lambda h: _state.update(hook=h)
        import antenv
        sys.modules["antenv.axon_hooks"] = mod
        antenv.axon_hooks = mod
        return True
    except Exception:
        return False


def _perm_node(g):
    g = np.asarray(g, np.int64)
    is_item = g >= U
    u = np.minimum(g, U - 1)
    ku = u // UPC
    pu = ku * NLOC + (u - ku * UPC)
    it = np.maximum(g - U, 0)
    ki = it // IPC
    pi = ki * NLOC + UPAD + (it - ki * IPC)
    return np.where(is_item, pi, pu)


def _wrap_idx(a):
    return np.tile(a.reshape(-1, 16).T, (8, 1)).astype(np.int16)


def _calls(slots):
    out, off = [], 0
    while off < slots:
        c = min(CHUNK, slots - off)
        out.append((off, c))
        off += c
    return out


class _Plan:
    """Static spmm schedule, identical across cores.
    T[b][w] = columns for (bucket, window)."""
    def __init__(self, nbuck, nwin, dout, T):
        self.nbuck, self.nwin, self.dout, self.T = nbuck, nwin, dout, T
        self.bslots = [int(T[b].sum()) * 128 for b in range(nbuck)]
        self.calls = [_calls(s) for s in self.bslots]
        self.colwin = [np.repeat(np.arange(nwin), T[b]) for b in range(nbuck)]


def _schedule(core, dl, cp, vv, nbuck, nwin, dout):
    """Build shared schedule + per-core slot streams."""
    key = (cp // BUCKET) * nwin + (dl // 128)
    counts = np.zeros((NC, nbuck * nwin), np.int64)
    for k in range(NC):
        counts[k] = np.bincount(key[core == k], minlength=nbuck * nwin)
    T = -(-counts.max(0) // 128)          # ceil(max/128), may be 0
    T = T.reshape(nbuck, nwin)
    plan = _Plan(nbuck, nwin, dout, T)
    gstart = np.concatenate([[0], np.cumsum(T.reshape(-1) * 128)])[:-1]

    streams = []
    for k in range(NC):
        m = core == k
        dlk, cpk, vvk = dl[m], cp[m], vv[m]
        keyk = (cpk // BUCKET) * nwin + (dlk // 128)
        o = np.argsort(keyk, kind="stable")
        dlk, cpk, vvk, keyk = dlk[o], cpk[o], vvk[o], keyk[o]
        ck = counts[k]
        ranks = np.arange(len(keyk)) - np.repeat(np.concatenate([[0], np.cumsum(ck)])[:-1], ck)
        slots = gstart[keyk] + ranks
        tot = int(T.sum()) * 128
        idx = np.zeros(tot, np.int16)
        dstr = np.zeros(tot, np.float32)
        val = np.zeros(tot, np.float32)
        idx[slots] = (cpk % BUCKET).astype(np.int16)
        dstr[slots] = (dlk % 128).astype(np.float32)
        val[slots] = vvk
        streams.append((idx, dstr, val))
    return plan, streams


def _emit_spmm(nc, sb, ps, plan, table_ap, table_rows, ix_dram, dv_dram, vl_dram,
               out_sb, iota0, tagp, gat_bufs=None):
    for b in range(plan.nbuck):
        if plan.bslots[b] == 0:
            continue
        base_rows = b * BUCKET
        nrows = min(BUCKET, table_rows - base_rows)
        colwin = plan.colwin[b]
        ncol_total = plan.bslots[b] // 128
        first = {}
        last = {}
        for j in range(ncol_total):
            w = int(colwin[j])
            first.setdefault(w, j)
            last[w] = j
        ptile = {}
        for ci, (off, csz) in enumerate(plan.calls[b]):
            ccols = csz // 128
            ix = sb.tile([128, CHUNK // 16], mybir.dt.int16, tag="ix")
            nc.sync.dma_start(ix[:, :csz // 16], ix_dram[b, ci, :, :csz // 16])
            gat = sb.tile([128, CHUNK // 128, plan.dout], mybir.dt.float32, tag="g", bufs=gat_bufs)
            nc.gpsimd.dma_gather(gat[:, :ccols, :],
                                 table_ap[base_rows:base_rows + nrows, :],
                                 ix[:, :csz // 16], csz, csz, plan.dout,
                                 single_packet=False)
            dv = sb.tile([128, CHUNK // 128], mybir.dt.float32, tag="d")
            vl = sb.tile([128, CHUNK // 128], mybir.dt.float32, tag="v")
            nc.sync.dma_start(dv[:, :ccols], dv_dram[b, ci, :, :ccols])
            nc.sync.dma_start(vl[:, :ccols], vl_dram[b, ci, :, :ccols])
            NSB = 32
            for sb0 in range(0, ccols, NSB):
                nn = min(NSB, ccols - sb0)
                Sb = sb.tile([128, NSB, 128], mybir.dt.float32, tag="S", bufs=2,
                             name=f"Sb{tagp}_{b}_{ci}_{sb0}")
                iota_b = iota0[:].unsqueeze(1).broadcast_to([128, nn, 128])
                dvb = dv[:, sb0:sb0 + nn].unsqueeze(2).broadcast_to([128, nn, 128])
                vlb = vl[:, sb0:sb0 + nn].unsqueeze(2).broadcast_to([128, nn, 128])
                nc.vector.tensor_tensor(Sb[:, :nn, :], iota_b, dvb,
                                        mybir.AluOpType.is_equal)
                nc.vector.tensor_tensor(Sb[:, :nn, :], Sb[:, :nn, :], vlb,
                                        mybir.AluOpType.mult)
                for lc in range(sb0, sb0 + nn):
                    j = off // 128 + lc
                    w = int(colwin[j])
                    if w not in ptile:
                        ptile[w] = ps.tile([128, plan.dout], mybir.dt.float32,
                                           tag="p", name=f"pw{tagp}_{w}")
                    nc.tensor.matmul(ptile[w][:], lhsT=Sb[:, lc - sb0, :],
                                     rhs=gat[:, lc, :],
                                     start=(first[w] == j), stop=(last[w] == j))
                    if last[w] == j:
                        nc.vector.tensor_add(out_sb[:, w, :], out_sb[:, w, :],
                                             ptile[w][:])
                        del ptile[w]
        assert not ptile


def _build_and_run(in_maps, adj_plan, hv_plan, hu_plan):
    global last_exec_wall
    nc = bacc.Bacc("TRN2", target_bir_lowering=False, debug=False, num_devices=NC)
    f32, i16 = mybir.dt.float32, mybir.dt.int16

    x0 = nc.dram_tensor("x0", [XROWS, D], f32, kind="ExternalInput")
    x0sh = nc.dram_tensor("x0sh", [128, NWIN, D], f32, kind="ExternalInput")
    item_aug = nc.dram_tensor("item_aug", [ITEM_AUG_ROWS, 128], f32, kind="ExternalInput")

    def edge_inputs(pfx, plan):
        ncall = max(len(c) for c in plan.calls)
        ixs = nc.dram_tensor(f"{pfx}_ix", [plan.nbuck, ncall, 128, CHUNK // 16], i16,
                             kind="ExternalInput")
        dvs = nc.dram_tensor(f"{pfx}_dv", [plan.nbuck, ncall, 128, CHUNK // 128], f32,
                             kind="ExternalInput")
        vls = nc.dram_tensor(f"{pfx}_vl", [plan.nbuck, ncall, 128, CHUNK // 128], f32,
                             kind="ExternalInput")
        return ixs, dvs, vls

    adj_io = edge_inputs("adj", adj_plan)
    hv_io = edge_inputs("hv", hv_plan)
    hu_io = edge_inputs("hu", hu_plan)

    out_u = nc.dram_tensor("out_u", [128, UWIN, D], f32, kind="ExternalOutput")
    out_i = nc.dram_tensor("out_i", [128, IPAD // 128, D], f32, kind="ExternalOutput")

    xf1 = nc.dram_tensor("xf1", [XROWS, D], f32, addr_space="Shared")
    xf2 = nc.dram_tensor("xf2", [XROWS, D], f32, addr_space="Shared")
    ebounce = nc.dram_tensor("ebounce", [NLOC, D], f32)
    bbounce = nc.dram_tensor("bbounce", [BPAD, 128], f32)
    bfull = nc.dram_tensor("bfull", [BROWS, 128], f32, addr_space="Shared")

    with tile.TileContext(nc) as tc:
        with tc.tile_pool(name="persist", bufs=1) as persist, \
             tc.tile_pool(name="ps", bufs=8, space="PSUM") as ps:
            iota0 = persist.tile([128, 128], f32)
            nc.gpsimd.iota(iota0[:], pattern=[[1, 128]], base=0, channel_multiplier=0,
                           allow_small_or_imprecise_dtypes=True)
            acc = persist.tile([128, NWIN, D], f32)
            nc.sync.dma_start(acc[:], x0sh[:])

            with tc.tile_pool(name="adjbig", bufs=1) as adjbig, \
                 tc.tile_pool(name="sb1", bufs=4) as sb:
                out_sb = adjbig.tile([128, NWIN, D], f32)
                for li, srct in enumerate([x0, xf1, xf2]):
                    nc.vector.memset(out_sb[:], 0.0)
                    _emit_spmm(nc, sb, ps, adj_plan, srct, XROWS, *adj_io,
                               out_sb, iota0, f"a{li}")
                    nc.vector.tensor_add(acc[:], acc[:], out_sb[:])
                    if li < 2:
                        nc.sync.dma_start(ebounce.rearrange("(n p) d -> p n d", p=128),
                                          out_sb[:])
                        nc.gpsimd.collective_compute(
                            "AllGather", mybir.AluOpType.bypass,
                            replica_groups=[list(range(NC))],
                            ins=[ebounce[:].opt()], outs=[[xf1, xf2][li][:].opt()])

            with tc.tile_pool(name="hubig", bufs=1) as hubig, \
                 tc.tile_pool(name="sb2", bufs=3) as sb:
                # ---- hv: items -> bicliques (aug col 64 = degree)
                bic = hubig.tile([128, BPAD // 128, 128], f32)
                nc.vector.memset(bic[:], 0.0)
                _emit_spmm(nc, sb, ps, hv_plan, item_aug, ITEM_AUG_ROWS, *hv_io,
                           bic, iota0, "hv", gat_bufs=2)
                nbr = BPAD // 128
                deg = sb.tile([128, nbr], f32, tag="bdeg")
                rec = sb.tile([128, nbr], f32, tag="brec")
                nc.vector.scalar_tensor_tensor(deg[:], bic[:, :, 64], 0.0, bic[:, :, 64],
                                               mybir.AluOpType.is_equal,
                                               mybir.AluOpType.add)
                nc.vector.reciprocal(rec[:], deg[:])
                for r in range(nbr):
                    nc.vector.tensor_scalar_mul(bic[:, r, 0:64], bic[:, r, 0:64],
                                                rec[:, r:r + 1])
                nc.vector.memset(bic[:, :, 64:65], 1.0)
                nc.vector.memset(bic[:, :, 65:128], 0.0)
                nc.sync.dma_start(bbounce.rearrange("(n p) d -> p n d", p=128), bic[:])
                nc.gpsimd.collective_compute(
                    "AllGather", mybir.AluOpType.bypass,
                    replica_groups=[list(range(NC))],
                    ins=[bbounce[:].opt()], outs=[bfull[:].opt()])

                # ---- hu: bicliques -> users
                ul = hubig.tile([128, UWIN, 128], f32)
                nc.vector.memset(ul[:], 0.0)
                _emit_spmm(nc, sb, ps, hu_plan, bfull, BROWS, *hu_io,
                           ul, iota0, "hu", gat_bufs=2)
                udeg = sb.tile([128, UWIN], f32, tag="udeg")
                urec = sb.tile([128, UWIN], f32, tag="urec")
                nc.vector.scalar_tensor_tensor(udeg[:], ul[:, :, 64], 0.0, ul[:, :, 64],
                                               mybir.AluOpType.is_equal,
                                               mybir.AluOpType.add)
                nc.vector.reciprocal(urec[:], udeg[:])
                nc.vector.tensor_scalar(acc[:], acc[:], 0.25, None,
                                        mybir.AluOpType.mult)
                for r in range(UWIN):
                    nc.vector.tensor_scalar(ul[:, r, 0:64], ul[:, r, 0:64],
                                            urec[:, r:r + 1], None,
                                            mybir.AluOpType.mult)
                    nc.vector.tensor_add(ul[:, r, 0:64], ul[:, r, 0:64],
                                         acc[:, r, :])
                nc.sync.dma_start(out_u[:], ul[:, :, 0:64])
                nc.sync.dma_start(out_i[:], acc[:, UWIN:NWIN, :])
    nc.compile()
    global last_exec_ns, last_res
    import os as _os
    trace = _os.environ.get("BASS_PROFILE", "0") == "1" and _install_ntff_hook()
    t0 = time.time()
    res = run_bass_kernel_spmd(nc, in_maps, list(range(NC)), trace=trace)
    last_exec_wall = time.time() - t0
    if trace:
        last_exec_ns = res.exec_time_ns
        last_res = res
    return res


def _pack_inputs(stream, plan):
    ncall = max(len(c) for c in plan.calls)
    ix = np.zeros((plan.nbuck, ncall, 128, CHUNK // 16), np.int16)
    dv = np.zeros((plan.nbuck, ncall, 128, CHUNK // 128), np.float32)
    vl = np.zeros((plan.nbuck, ncall, 128, CHUNK // 128), np.float32)
    idx, dstr, val = stream
    boff = 0
    for b in range(plan.nbuck):
        for ci, (off, csz) in enumerate(plan.calls[b]):
            s = boff + off
            ix[b, ci, :, :csz // 16] = _wrap_idx(idx[s:s + csz])
            dv[b, ci, :, :csz // 128] = dstr[s:s + csz].reshape(-1, 128).T
            vl[b, ci, :, :csz // 128] = val[s:s + csz].reshape(-1, 128).T
        boff += plan.bslots[b]
    return ix, dv, vl


def kernel(user_emb, item_emb, adj_val, hv_val, hu_val,
           adj_row, adj_col, hv_row, hv_col, hu_row, hu_col):
    user_emb = np.asarray(user_emb, np.float32)
    item_emb = np.asarray(item_emb, np.float32)
    adj_val = np.asarray(adj_val, np.float32)
    hv_val = np.asarray(hv_val, np.float32)
    hu_val = np.asarray(hu_val, np.float32)
    adj_row = np.asarray(adj_row, np.int64)
    adj_col = np.asarray(adj_col, np.int64)
    hv_row = np.asarray(hv_row, np.int64)
    hv_col = np.asarray(hv_col, np.int64)
    hu_row = np.asarray(hu_row, np.int64)
    hu_col = np.asarray(hu_col, np.int64)

    x0 = np.zeros((XROWS, D), np.float32)
    allp = _perm_node(np.arange(U + I))
    x0[allp[:U]] = user_emb
    x0[allp[U:]] = item_emb

    item_aug = np.zeros((ITEM_AUG_ROWS, 128), np.float32)
    item_aug[:I, :64] = item_emb
    item_aug[:I, 64] = 1.0

    def core_of_node(g):
        g = np.asarray(g, np.int64)
        return np.where(g < U, np.minimum(g // UPC, NC - 1),
                        np.minimum((g - U) // IPC, NC - 1))

    def local_of_node(g):
        g = np.asarray(g, np.int64)
        k = core_of_node(g)
        return np.where(g < U, g - k * UPC, UPAD + (g - U) - k * IPC)

    adj_core = core_of_node(adj_row)
    adj_dl = local_of_node(adj_row)
    adj_cp = _perm_node(adj_col)
    hv_core = hv_row // BPC
    hv_dl = hv_row - hv_core * BPC
    hu_core = hu_row // UPC
    hu_dl = hu_row - hu_core * UPC
    hu_cp = hu_col // BPC * BPAD + hu_col % BPC

    adj_plan, adj_streams = _schedule(adj_core, adj_dl, adj_cp, adj_val, 5, NWIN, D)
    hv_plan, hv_streams = _schedule(hv_core, hv_dl, hv_col, hv_val, 2, BPAD // 128, 128)
    hu_plan, hu_streams = _schedule(hu_core, hu_dl, hu_cp, hu_val, 1, UWIN, 128)

    in_maps = []
    for k in range(NC):
        a = _pack_inputs(adj_streams[k], adj_plan)
        v = _pack_inputs(hv_streams[k], hv_plan)
        u = _pack_inputs(hu_streams[k], hu_plan)
        x0sh = x0[k * NLOC:(k + 1) * NLOC].reshape(NWIN, 128, D).transpose(1, 0, 2).copy()
        in_maps.append({
            "x0": x0, "x0sh": x0sh, "item_aug": item_aug,
            "adj_ix": a[0], "adj_dv": a[1], "adj_vl": a[2],
            "hv_ix": v[0], "hv_dv": v[1], "hv_vl": v[2],
            "hu_ix": u[0], "hu_dv": u[1], "hu_vl": u[2],
        })

    res = _build_and_run(in_maps, adj_plan, hv_plan, hu_plan)

    u_out = np.zeros((U, D), np.float32)
    i_out = np.zeros((I, D), np.float32)
    for k in range(NC):
        ou = res.results[k]["out_u"]
        oi = res.results[k]["out_i"]
        u_out[k * UPC:(k + 1) * UPC] = ou.transpose(1, 0, 2).reshape(UPAD, D)[:UPC]
        i_out[k * IPC:(k + 1) * IPC] = oi.transpose(1, 0, 2).reshape(IPAD, D)[:IPC]
    return u_out, i_out
